# revision 1
# baseline (speedup 1.0000x reference)
"""Trainium2 Bass kernel for nn_BigFanoutModel (100 tiny fanout matmuls + sum).

Math: out[k] = sum_{n,d} x[0,d] * matrices[n,d,k] == x @ (sum_n matrices[n]).
Shapes: x (1,4) f32, matrices (100,4,4) f32 -> out (4,) f32.

Total input is 6.4KB, so the problem is pure latency. Per the sharding hint
("too small to shard meaningfully"), the full inputs are replicated on all 8
cores; every core computes the full output with a minimal instruction chain
and core 0's result is returned. No collectives.

Per-core dataflow (engines: SP=sync DMA, ACT=scalar DMA, DVE=vector, PE):
  SP   A_sb[100,16] <- matrices, contiguous (100 rows x 64B)
  ACT  x_sb[1,4]    <- x                  (parallel HWDGE queue)
  DVE  ones[100,1]  <- memset 1.0
  PE   U[1,16]      <- ones.T @ A_sb      (contracts n=100 in one matmul)
  DVE  W[1,16]      <- U * x              (x broadcast along k via stride-0 AP)
  DVE  res[1,4]     <- sum over d of W    (strided view, reduce X)
  SP   out[4]       <- res, then wait for the write receipt

Implementation notes:
- Raw Bass (no Tile): the whole kernel is ~9 instructions; Tile's scheduler
  and its kernel-tail barrier only add overhead at this size.
- "Lean" Bass construction: the const-AP memsets and the init-time
  all-engine barrier emitted by Bass.__init__ are suppressed (nothing here
  uses the const pool, and the NEFF's runtime prologue already synchronizes
  the engines). No Block() wrapper -> no exit barrier.
- The DVE mul->reduce pair carries an explicit same-engine semaphore wait:
  DVE pipelines back-to-back instructions, so the reduce would otherwise
  read w_sb before the multiply's writes land (confirmed by the CoreSim
  race detector and by a wrong result on hardware).
- fp32 matmul runs as a LOW/HIGH dual pass on the PE; keeping the moving
  free dim at N=16 makes each pass ~185ns (vs ~850ns at N=400).
- Measured on trn2 (NTFF profile, first-to-last instruction): ~18.0-18.9us
  total, of which ~14us is the runtime-injected NEFF prologue/epilogue
  (engine start + sem-file reset, identical for any kernel here) and ~4us
  is this kernel's body (dominated by the two HBM round trips).
"""

import numpy as np

import concourse.bass as bass
import concourse.mybir as mybir
from concourse.bass_utils import run_bass_kernel_spmd

N_CORES = 8

_NC_CACHE = None


def _make_bass_lean():
    """Bass() without the const-AP memsets and init all-engine barrier."""
    orig_barrier = bass.Bass.all_engine_barrier
    orig_memset = bass.BassGpSimd.memset
    bass.Bass.all_engine_barrier = lambda self, **k: None
    bass.BassGpSimd.memset = lambda self, ap, c: None
    try:
        nc = bass.Bass(monotonic_sem_count=0)
    finally:
        bass.Bass.all_engine_barrier = orig_barrier
        bass.BassGpSimd.memset = orig_memset
    return nc


def _build_nc():
    nc = _make_bass_lean()
    x = nc.dram_tensor("x", [1, 4], mybir.dt.float32, kind="ExternalInput")
    m = nc.dram_tensor("matrices", [100, 4, 4], mybir.dt.float32, kind="ExternalInput")
    o = nc.dram_tensor("out", [4], mybir.dt.float32, kind="ExternalOutput")
    with (
        nc.semaphore("semA") as semA,
        nc.semaphore("semX") as semX,
        nc.semaphore("semO") as semO,
        nc.semaphore("c") as c,
        nc.sbuf_tensor("A_sb", [100, 16], mybir.dt.float32) as A_sb,
        nc.sbuf_tensor("ones_sb", [100, 1], mybir.dt.float32) as ones_sb,
        nc.sbuf_tensor("x_sb", [1, 4], mybir.dt.float32) as x_sb,
        nc.sbuf_tensor("w_sb", [1, 16], mybir.dt.float32) as w_sb,
        nc.sbuf_tensor("res_sb", [1, 4], mybir.dt.float32) as res_sb,
        nc.psum_tensor("u_ps", [1, 16], mybir.dt.float32) as u_ps,
    ):
        # SP: matrices (the long-pole transfer); ACT: x in parallel.
        nc.sync.dma_start(
            bass.AP(A_sb, 0, [[16, 100], [1, 16]]),
            bass.AP(m, 0, [[16, 100], [1, 16]]),
        ).then_inc(semA, 16)
        nc.scalar.dma_start(
            bass.AP(x_sb, 0, [[4, 1], [1, 4]]),
            bass.AP(x, 0, [[4, 1], [1, 4]]),
        ).then_inc(semX, 16)

        # DVE: ones vector for the n-contraction.
        nc.vector.memset(bass.AP(ones_sb, 0, [[1, 100], [1, 1]]), 1.0).then_inc(c, 1)

        # PE: U[1,16] = ones.T @ A  == sum_n matrices[n], flattened (d,k).
        nc.tensor.wait_ge(c, 1)
        nc.tensor.wait_ge(semA, 16)
        nc.tensor.matmul(
            bass.AP(u_ps, 0, [[16, 1], [1, 16]]),
            bass.AP(ones_sb, 0, [[1, 100], [1, 1]]),
            bass.AP(A_sb, 0, [[16, 100], [1, 16]]),
        ).then_inc(c, 1)

        # DVE: W[d,k] = U[d,k] * x[d]; then res[k] = sum_d W[d,k].
        # semX first: x's receipt lands ~1us before the matmul finishes, so
        # this wait clears while PE is still busy; waiting on c last means
        # the multiply issues immediately after the matmul's increment.
        nc.vector.wait_ge(semX, 16)
        nc.vector.wait_ge(c, 2)
        nc.vector.tensor_mul(
            bass.AP(w_sb, 0, [[16, 1], [4, 4], [1, 4]]),
            bass.AP(u_ps, 0, [[16, 1], [4, 4], [1, 4]]),
            bass.AP(x_sb, 0, [[4, 1], [1, 4], [0, 4]]),
        ).then_inc(c, 1)
        nc.vector.wait_ge(c, 3)  # same-engine pipeline hazard on w_sb
        nc.vector.reduce_sum(
            out=bass.AP(res_sb, 0, [[4, 1], [1, 4]]),
            in_=bass.AP(w_sb, 0, [[16, 1], [1, 4], [4, 4]]),
            axis=mybir.AxisListType.X,
        ).then_inc(c, 1)

        # SP: out, with an explicit completion wait. (A fire-and-forget
        # variant saves ~1us but races the runtime's end-of-NEFF semaphore
        # reset; an NRT_EXEC_UNIT_UNRECOVERABLE was observed under repeated
        # executions without this wait, so keep it.)
        nc.sync.wait_ge(c, 4)
        nc.sync.dma_start(
            bass.AP(o, 0, [[1, 4]]),
            bass.AP(res_sb, 0, [[4, 1], [1, 4]]),
        ).then_inc(semO, 16)
        nc.sync.wait_ge(semO, 16)
    return nc


def _get_nc():
    global _NC_CACHE
    if _NC_CACHE is None:
        _NC_CACHE = _build_nc()
    return _NC_CACHE


def _run(x, matrices, **kwargs):
    """Uncached path (kept for test harnesses that want BassKernelResults)."""
    nc = _get_nc()
    in_map = {
        "x": np.ascontiguousarray(x, dtype=np.float32),
        "matrices": np.ascontiguousarray(matrices, dtype=np.float32),
    }
    in_maps = [in_map for _ in range(N_CORES)]
    return run_bass_kernel_spmd(nc, in_maps, list(range(N_CORES)), **kwargs)


def kernel(x, matrices):
    # Fresh dispatch per call (the ecosystem-default run_bass_kernel_spmd
    # path). Each call executes the NEFF as a first execution, which has a
    # ~8us faster runtime prologue than re-executing a cached executable
    # (re-execution repeats the engine-state TENSOR_LOAD round). The
    # compiled NEFF itself comes from the on-disk neuron compile cache, so
    # per-call overhead is only the PJRT trace+load (~0.7s wall).
    res = _run(x, matrices)
    return np.asarray(res.results[0]["out"], dtype=np.float32).reshape(4)



# revision 2
# speedup vs baseline: 1.0481x; 1.0481x over previous
"""Trainium2 Bass kernel for nn_BigFanoutModel (100 tiny fanout matmuls + sum).

Math: out[k] = sum_{n,d} x[0,d] * matrices[n,d,k] == x @ (sum_n matrices[n]).
Shapes: x (1,4) f32, matrices (100,4,4) f32 -> out (4,) f32.

Total input is 6.4KB, so the problem is pure latency. Per the sharding hint
("too small to shard meaningfully"), the full inputs are replicated on all 8
cores; every core computes the full output with a minimal instruction chain
and core 0's result is returned. No collectives.

Per-core dataflow (engines: SP=sync DMA, ACT=scalar DMA, DVE=vector, PE):
  ACT  A_sb[100,16] <- matrices, contiguous (100 rows x 64B)
  SP   x_sb[1,4]    <- x                  (parallel HWDGE queue)
  DVE  ones[100,1]  <- memset 1.0
  PE   U[1,16]      <- ones.T @ A_sb      (contracts n=100 in one matmul)
  DVE  W[1,16]      <- U * x              (x broadcast along k via stride-0 AP)
  DVE  res[1,4]     <- sum over d of W    (strided view, reduce X)
  SP   out[4]       <- res, fire-and-forget (no completion wait)

Timeline facts (measured via NTFF on trn2; instruction-span convention):
- ~6.4us runtime-injected prologue before any body instruction can issue
  (engine start + host doorbell wait on $E[4] + per-engine TENSOR_LOAD +
  barriers + register init), and ~7.0us epilogue after the last body
  instruction (an all-engine barrier, then each engine serially resets its
  ~51-semaphore slice of the sem file at ~115ns/sem, then a final barrier +
  NOTIFY). Both are emitted by the runtime at NEFF load, identical for any
  kernel, and account for ~13.5us of the measured span.
- HWDGE DMA receipt latency is ~2.2-2.4us: ~25ns seq + ~630ns HWDGE issue +
  ~650-780ns DGE delay + transfer + ~900ns completion-semaphore propagation
  (TRN2Spec SEM_PROP_DMA_OVERHEAD_NS). The 16 hardware queues complete a
  dynamic DMA with 16 separate +1 increments spread over ~0.5-1.0us.
- The epilogue starts only when ALL engines finish their body, so the
  measured span = (last engine's last instruction) + ~7.0us.

Optimizations over the first working version (~19.0us -> ~18.0us):
- matrices (the long-pole transfer) moved to ACT, whose injected prologue
  finishes ~100-600ns before SP's (SP sporadically runs a ~700ns DRAIN); x
  moved to SP, where its receipt (~2us) still beats the DVE multiply by
  ~0.5us.
- The output DMA is fire-and-forget: no engine waits for its completion
  receipt, which removes a ~1.1us DMA round-trip + sem-propagation tail from
  the span. Walrus codegen requires DGE sync info, so the DMA still
  increments semO -- but the sem INDEX matters: the runtime epilogue resets
  S[105..155] on GpSimd and S[156..206] on Vector top-down at ~115ns/sem
  starting right after the post-body barrier. Bass numbers user sems from
  154. A completion increment landing AFTER its sem's reset leaves a stale
  nonzero semaphore for the next execution (observed as
  NRT_EXEC_UNIT_UNRECOVERABLE in a previous session, where semO sat at
  S[157] = position 1 of Vector's slice, reset ~250ns into the epilogue,
  while the DMA completion lands ~1.7us after issue). semO is therefore
  allocated FIRST so it gets S[154] = position 49 of GpSimd's slice, reset
  ~5.6us into the epilogue -- a ~4.5us safety margin. Verified stable over
  ~100 executions across multiple sessions.

Approaches measured as no better (kept out):
- single_packet DMAs, pruned DMA-queue declarations, running on 1 core
  instead of 8, packing x+ones into the matrices upload: all within the
  +/-200ns session noise.
- Dual-issuing the matrices DMA on both HWDGE engines into the same tile
  with one semaphore is UNSOUND: the 16-increment threshold can fire from
  the two copies' interleaved partial completions (produced a wrong first
  run), and there is no OR-wait across two semaphores.
- SWDGE prepared-descriptor scatter-add with trigger_dma (would cut the
  ~650ns tail DMA issue): InstDMAScatterAddAnt / InstPseudoReloadLibraryIndex
  fail this walrus build's codegen ("ISA wrong length").
Session-to-session drift of up to +2.5us (slower engine clocks: the same
NEFF shows sem-reset spacing 115ns vs 138ns) dominates remaining variance.

Implementation notes:
- Raw Bass (no Tile): the whole kernel is ~11 instructions; Tile's scheduler
  and its kernel-tail barrier only add overhead at this size.
- "Lean" Bass construction: the const-AP memsets and the init-time
  all-engine barrier emitted by Bass.__init__ are suppressed (nothing here
  uses the const pool, and the NEFF's runtime prologue already synchronizes
  the engines). No Block() wrapper -> no exit barrier.
- The DVE mul->reduce pair carries an explicit same-engine semaphore wait:
  DVE pipelines back-to-back instructions under relaxed ordering, so the
  reduce would otherwise read w_sb before the multiply's writes land.
- fp32 matmul runs as a LOW/HIGH dual pass on the PE; keeping the moving
  free dim at N=16 makes the pair ~240ns total.
"""

import numpy as np

import concourse.bass as bass
import concourse.mybir as mybir
from concourse.bass_utils import run_bass_kernel_spmd

N_CORES = 8

_NC_CACHE = None


def _make_bass_lean():
    """Bass() without the const-AP memsets and init all-engine barrier."""
    orig_barrier = bass.Bass.all_engine_barrier
    orig_memset = bass.BassGpSimd.memset
    bass.Bass.all_engine_barrier = lambda self, **k: None
    bass.BassGpSimd.memset = lambda self, ap, c: None
    try:
        nc = bass.Bass(monotonic_sem_count=0)
    finally:
        bass.Bass.all_engine_barrier = orig_barrier
        bass.BassGpSimd.memset = orig_memset
    return nc


def _build_nc():
    nc = _make_bass_lean()
    x = nc.dram_tensor("x", [1, 4], mybir.dt.float32, kind="ExternalInput")
    m = nc.dram_tensor("matrices", [100, 4, 4], mybir.dt.float32, kind="ExternalInput")
    o = nc.dram_tensor("out", [4], mybir.dt.float32, kind="ExternalOutput")
    with (
        # semO FIRST: the fire-and-forget out-DMA's completion sem must get
        # index 154 (late-reset position in the runtime's epilogue sweep) --
        # see module docstring.
        nc.semaphore("semO") as semO,
        nc.semaphore("semA") as semA,
        nc.semaphore("semX") as semX,
        nc.semaphore("c") as c,
        nc.sbuf_tensor("A_sb", [100, 16], mybir.dt.float32) as A_sb,
        nc.sbuf_tensor("ones_sb", [100, 1], mybir.dt.float32) as ones_sb,
        nc.sbuf_tensor("x_sb", [1, 4], mybir.dt.float32) as x_sb,
        nc.sbuf_tensor("w_sb", [1, 16], mybir.dt.float32) as w_sb,
        nc.sbuf_tensor("res_sb", [1, 4], mybir.dt.float32) as res_sb,
        nc.psum_tensor("u_ps", [1, 16], mybir.dt.float32) as u_ps,
    ):
        # ACT: matrices (the long-pole transfer); SP: x in parallel.
        nc.scalar.dma_start(
            bass.AP(A_sb, 0, [[16, 100], [1, 16]]),
            bass.AP(m, 0, [[16, 100], [1, 16]]),
        ).then_inc(semA, 16)
        nc.sync.dma_start(
            bass.AP(x_sb, 0, [[4, 1], [1, 4]]),
            bass.AP(x, 0, [[4, 1], [1, 4]]),
        ).then_inc(semX, 16)

        # DVE: ones vector for the n-contraction.
        nc.vector.memset(bass.AP(ones_sb, 0, [[1, 100], [1, 1]]), 1.0).then_inc(c, 1)

        # PE: U[1,16] = ones.T @ A  == sum_n matrices[n], flattened (d,k).
        nc.tensor.wait_ge(c, 1)
        nc.tensor.wait_ge(semA, 16)
        nc.tensor.matmul(
            bass.AP(u_ps, 0, [[16, 1], [1, 16]]),
            bass.AP(ones_sb, 0, [[1, 100], [1, 1]]),
            bass.AP(A_sb, 0, [[16, 100], [1, 16]]),
        ).then_inc(c, 1)

        # DVE: W[d,k] = U[d,k] * x[d]; then res[k] = sum_d W[d,k].
        # semX first: x's receipt lands well before the matmul finishes, so
        # this wait clears while PE is still busy; waiting on c last means
        # the multiply issues immediately after the matmul's increment.
        nc.vector.wait_ge(semX, 16)
        nc.vector.wait_ge(c, 2)
        nc.vector.tensor_mul(
            bass.AP(w_sb, 0, [[16, 1], [4, 4], [1, 4]]),
            bass.AP(u_ps, 0, [[16, 1], [4, 4], [1, 4]]),
            bass.AP(x_sb, 0, [[4, 1], [1, 4], [0, 4]]),
        ).then_inc(c, 1)
        nc.vector.wait_ge(c, 3)  # same-engine pipeline hazard on w_sb
        nc.vector.reduce_sum(
            out=bass.AP(res_sb, 0, [[4, 1], [1, 4]]),
            in_=bass.AP(w_sb, 0, [[16, 1], [1, 4], [4, 4]]),
            axis=mybir.AxisListType.X,
        ).then_inc(c, 1)

        # SP: out, fire-and-forget. The 16B write + semO increments land
        # ~1.7us after issue, during the runtime epilogue; S[154] is reset
        # ~5.6us into the epilogue, so the increments always land first.
        nc.sync.wait_ge(c, 4)
        nc.sync.dma_start(
            bass.AP(o, 0, [[1, 4]]),
            bass.AP(res_sb, 0, [[4, 1], [1, 4]]),
        ).then_inc(semO, 16)
    return nc


def _get_nc():
    global _NC_CACHE
    if _NC_CACHE is None:
        _NC_CACHE = _build_nc()
    return _NC_CACHE


def _run(x, matrices, **kwargs):
    """Uncached path (kept for test harnesses that want BassKernelResults)."""
    nc = _get_nc()
    in_map = {
        "x": np.ascontiguousarray(x, dtype=np.float32),
        "matrices": np.ascontiguousarray(matrices, dtype=np.float32),
    }
    in_maps = [in_map for _ in range(N_CORES)]
    return run_bass_kernel_spmd(nc, in_maps, list(range(N_CORES)), **kwargs)


def kernel(x, matrices):
    # Fresh dispatch per call (the ecosystem-default run_bass_kernel_spmd
    # path). Each call executes the NEFF as a first execution, which has a
    # faster runtime prologue than re-executing a cached executable. The
    # compiled NEFF itself comes from the on-disk neuron compile cache, so
    # per-call overhead is only the PJRT trace+load (~0.7s wall).
    res = _run(x, matrices)
    return np.asarray(res.results[0]["out"], dtype=np.float32).reshape(4)


# revision 3
# speedup vs baseline: 1.0500x; 1.0018x over previous
"""Trainium2 Bass kernel for nn_BigFanoutModel (100 tiny fanout matmuls + sum).

Math: out[k] = sum_{n,d} x[0,d] * matrices[n,d,k] == x @ (sum_n matrices[n]).
Shapes: x (1,4) f32, matrices (100,4,4) f32 -> out (4,) f32.

Total input is 6.4KB, so the problem is pure latency. Per the sharding hint
("too small to shard meaningfully"), the full inputs are replicated on all 8
cores; every core computes the full output with a minimal instruction chain
and core 0's result is returned. No collectives.

Per-core dataflow (engines: SP=sync DMA, ACT=scalar DMA, DVE=vector, PE):
  ACT  A_sb[100,16] <- matrices, contiguous (100 rows x 64B)
  SP   x_sb[1,4]    <- x                  (parallel HWDGE queue)
  DVE  ones[100,1]  <- memset 1.0
  PE   U[1,16]      <- ones.T @ A_sb      (contracts n=100 in one matmul)
  DVE  W[1,16]      <- U * x              (x broadcast along k via stride-0 AP)
  DVE  res[1,4]     <- sum over d of W    (strided view, reduce X)
  SP   out[4]       <- res, fire-and-forget (no completion wait)

Timeline facts (measured via NTFF on trn2; instruction-span convention):
- ~6.4us runtime-injected prologue before any body instruction can issue
  (engine start + host doorbell wait on $E[4] + per-engine TENSOR_LOAD +
  barriers + register init), and ~7.0us epilogue after the last body
  instruction (an all-engine barrier, then each engine serially resets its
  ~51-semaphore slice of the sem file at ~115ns/sem, then a final barrier +
  NOTIFY). Both are emitted by the runtime at NEFF load, identical for any
  kernel, and account for ~13.5us of the measured span.
- HWDGE DMA receipt latency is ~2.2-2.4us: ~25ns seq + ~630ns HWDGE issue +
  ~650-780ns DGE delay + transfer + ~900ns completion-semaphore propagation
  (TRN2Spec SEM_PROP_DMA_OVERHEAD_NS). The 16 hardware queues complete a
  dynamic DMA with 16 separate +1 increments spread over ~0.5-1.0us.
- The epilogue starts only when ALL engines finish their body, so the
  measured span = (last engine's last instruction) + ~7.0us.

Optimizations over the first working version (~19.0us -> ~18.0us):
- matrices (the long-pole transfer) moved to ACT, whose injected prologue
  finishes ~100-600ns before SP's (SP sporadically runs a ~700ns DRAIN); x
  moved to SP, where its receipt (~2us) still beats the DVE multiply by
  ~0.5us.
- The output DMA is fire-and-forget: no engine waits for its completion
  receipt, which removes a ~1.1us DMA round-trip + sem-propagation tail from
  the span. Walrus codegen requires DGE sync info, so the DMA still
  increments semO -- but the sem INDEX matters: the runtime epilogue resets
  S[105..155] on GpSimd and S[156..206] on Vector top-down at ~115ns/sem
  starting right after the post-body barrier. Bass numbers user sems from
  154. A completion increment landing AFTER its sem's reset leaves a stale
  nonzero semaphore for the next execution (observed as
  NRT_EXEC_UNIT_UNRECOVERABLE in a previous session, where semO sat at
  S[157] = position 1 of Vector's slice, reset ~250ns into the epilogue,
  while the DMA completion lands ~1.7us after issue). semO is therefore
  allocated FIRST so it gets S[154] = position 49 of GpSimd's slice, reset
  ~5.6us into the epilogue -- a ~4.5us safety margin. Verified stable over
  ~100 executions across multiple sessions.

Approaches measured as no better (kept out):
- pruned DMA-queue declarations, running on 1 core instead of 8, packing
  x+ones into the matrices upload: all within the +/-200ns session noise.
  (single_packet=True on the DMAs won both direct A/B sessions by
  ~150-250ns and is kept, though it does not change the 16-increment
  completion pattern.)
- Dual-issuing the matrices DMA on both HWDGE engines into the same tile
  with one semaphore is UNSOUND: the 16-increment threshold can fire from
  the two copies' interleaved partial completions (produced a wrong first
  run), and there is no OR-wait across two semaphores.
- SWDGE prepared-descriptor scatter-add with trigger_dma (would cut the
  ~650ns tail DMA issue): InstDMAScatterAddAnt / InstPseudoReloadLibraryIndex
  fail this walrus build's codegen ("ISA wrong length").
Session-to-session drift of up to +2.5us (slower engine clocks: the same
NEFF shows sem-reset spacing 115ns vs 138ns) dominates remaining variance.

Implementation notes:
- Raw Bass (no Tile): the whole kernel is ~11 instructions; Tile's scheduler
  and its kernel-tail barrier only add overhead at this size.
- "Lean" Bass construction: the const-AP memsets and the init-time
  all-engine barrier emitted by Bass.__init__ are suppressed (nothing here
  uses the const pool, and the NEFF's runtime prologue already synchronizes
  the engines). No Block() wrapper -> no exit barrier.
- The DVE mul->reduce pair carries an explicit same-engine semaphore wait:
  DVE pipelines back-to-back instructions under relaxed ordering, so the
  reduce would otherwise read w_sb before the multiply's writes land.
- fp32 matmul runs as a LOW/HIGH dual pass on the PE; keeping the moving
  free dim at N=16 makes the pair ~240ns total.
"""

import numpy as np

import concourse.bass as bass
import concourse.mybir as mybir
from concourse.bass_utils import run_bass_kernel_spmd

N_CORES = 8

_NC_CACHE = None


def _make_bass_lean():
    """Bass() without the const-AP memsets and init all-engine barrier."""
    orig_barrier = bass.Bass.all_engine_barrier
    orig_memset = bass.BassGpSimd.memset
    bass.Bass.all_engine_barrier = lambda self, **k: None
    bass.BassGpSimd.memset = lambda self, ap, c: None
    try:
        nc = bass.Bass(monotonic_sem_count=0)
    finally:
        bass.Bass.all_engine_barrier = orig_barrier
        bass.BassGpSimd.memset = orig_memset
    return nc


def _build_nc():
    nc = _make_bass_lean()
    x = nc.dram_tensor("x", [1, 4], mybir.dt.float32, kind="ExternalInput")
    m = nc.dram_tensor("matrices", [100, 4, 4], mybir.dt.float32, kind="ExternalInput")
    o = nc.dram_tensor("out", [4], mybir.dt.float32, kind="ExternalOutput")
    with (
        # semO FIRST: the fire-and-forget out-DMA's completion sem must get
        # index 154 (late-reset position in the runtime's epilogue sweep) --
        # see module docstring.
        nc.semaphore("semO") as semO,
        nc.semaphore("semA") as semA,
        nc.semaphore("semX") as semX,
        nc.semaphore("c") as c,
        nc.sbuf_tensor("A_sb", [100, 16], mybir.dt.float32) as A_sb,
        nc.sbuf_tensor("ones_sb", [100, 1], mybir.dt.float32) as ones_sb,
        nc.sbuf_tensor("x_sb", [1, 4], mybir.dt.float32) as x_sb,
        nc.sbuf_tensor("w_sb", [1, 16], mybir.dt.float32) as w_sb,
        nc.sbuf_tensor("res_sb", [1, 4], mybir.dt.float32) as res_sb,
        nc.psum_tensor("u_ps", [1, 16], mybir.dt.float32) as u_ps,
    ):
        # ACT: matrices (the long-pole transfer); SP: x in parallel.
        nc.scalar.dma_start(
            bass.AP(A_sb, 0, [[16, 100], [1, 16]]),
            bass.AP(m, 0, [[16, 100], [1, 16]]),
            single_packet=True,
        ).then_inc(semA, 16)
        nc.sync.dma_start(
            bass.AP(x_sb, 0, [[4, 1], [1, 4]]),
            bass.AP(x, 0, [[4, 1], [1, 4]]),
            single_packet=True,
        ).then_inc(semX, 16)

        # DVE: ones vector for the n-contraction.
        nc.vector.memset(bass.AP(ones_sb, 0, [[1, 100], [1, 1]]), 1.0).then_inc(c, 1)

        # PE: U[1,16] = ones.T @ A  == sum_n matrices[n], flattened (d,k).
        nc.tensor.wait_ge(c, 1)
        nc.tensor.wait_ge(semA, 16)
        nc.tensor.matmul(
            bass.AP(u_ps, 0, [[16, 1], [1, 16]]),
            bass.AP(ones_sb, 0, [[1, 100], [1, 1]]),
            bass.AP(A_sb, 0, [[16, 100], [1, 16]]),
        ).then_inc(c, 1)

        # DVE: W[d,k] = U[d,k] * x[d]; then res[k] = sum_d W[d,k].
        # semX first: x's receipt lands well before the matmul finishes, so
        # this wait clears while PE is still busy; waiting on c last means
        # the multiply issues immediately after the matmul's increment.
        nc.vector.wait_ge(semX, 16)
        nc.vector.wait_ge(c, 2)
        nc.vector.tensor_mul(
            bass.AP(w_sb, 0, [[16, 1], [4, 4], [1, 4]]),
            bass.AP(u_ps, 0, [[16, 1], [4, 4], [1, 4]]),
            bass.AP(x_sb, 0, [[4, 1], [1, 4], [0, 4]]),
        ).then_inc(c, 1)
        nc.vector.wait_ge(c, 3)  # same-engine pipeline hazard on w_sb
        nc.vector.reduce_sum(
            out=bass.AP(res_sb, 0, [[4, 1], [1, 4]]),
            in_=bass.AP(w_sb, 0, [[16, 1], [1, 4], [4, 4]]),
            axis=mybir.AxisListType.X,
        ).then_inc(c, 1)

        # SP: out, fire-and-forget. The 16B write + semO increments land
        # ~1.7us after issue, during the runtime epilogue; S[154] is reset
        # ~5.6us into the epilogue, so the increments always land first.
        nc.sync.wait_ge(c, 4)
        nc.sync.dma_start(
            bass.AP(o, 0, [[1, 4]]),
            bass.AP(res_sb, 0, [[4, 1], [1, 4]]),
            single_packet=True,
        ).then_inc(semO, 16)
    return nc


def _get_nc():
    global _NC_CACHE
    if _NC_CACHE is None:
        _NC_CACHE = _build_nc()
    return _NC_CACHE


def _run(x, matrices, **kwargs):
    """Uncached path (kept for test harnesses that want BassKernelResults)."""
    nc = _get_nc()
    in_map = {
        "x": np.ascontiguousarray(x, dtype=np.float32),
        "matrices": np.ascontiguousarray(matrices, dtype=np.float32),
    }
    in_maps = [in_map for _ in range(N_CORES)]
    return run_bass_kernel_spmd(nc, in_maps, list(range(N_CORES)), **kwargs)


def kernel(x, matrices):
    # Fresh dispatch per call (the ecosystem-default run_bass_kernel_spmd
    # path). Each call executes the NEFF as a first execution, which has a
    # faster runtime prologue than re-executing a cached executable. The
    # compiled NEFF itself comes from the on-disk neuron compile cache, so
    # per-call overhead is only the PJRT trace+load (~0.7s wall).
    res = _run(x, matrices)
    return np.asarray(res.results[0]["out"], dtype=np.float32).reshape(4)
